# revision 10
# baseline (speedup 1.0000x reference)
"""MoE layer (top-2 routing, 16 experts) on 8 Trainium2 NeuronCores.

Strategy: tensor-parallel expert sharding (TP=2). Each expert's FFN is split
along the intermediate dimension F into two half-shards [D, F/2] / [F/2, D];
the 32 shards are laid out on an 8-core x 4-slot grid (slot capacities fixed
across cores for SPMD), sorted so slot s's capacity is the s-th order
statistic of the duplicated count list: C = (c1 + c5 + c9 + c13)/2
column-equivalents vs c1 + c9 for whole-expert placement -- less padding
waste from expert load imbalance. Every token is shipped to both shard cores
of each of its two experts; the two half-FFN partial outputs (silu applies
per-F-element, so each shard's silu(x @ W1h) @ W2h is an exact partial term)
are summed on the host during the combine.

All 4 slots' weights (16 MB bf16) stay resident in SBUF, so there is no
weight-slot recycling WAR chain: every weight DMA issues up front in
consumption order. Slot 0's W1 + first tokens stream on the sync HWDGE
queue, slot 0's W2 on the scalar HWDGE queue in parallel (startup is
DMA-descriptor-rate-bound, so two queues nearly double the early stream
rate); later slots trail on sync. Tokens prefetch one supertile ahead on the
gpsimd SWDGE queue, first prefetch fence-gated behind the first supertile's
last silu so it can't compete with the startup weight crunch.

Device layout keeps tokens on the matmul free dimension throughout (x is
shipped transposed, [D, tokens]) so no on-chip transposes are needed:
  mm1: A^T[f, tok] += W1h[d, f]^T-chunks (stationary) @ x^T[d, tok]
  silu on ScalarE, PSUM -> SBUF (bf16)
  mm2: y^T[d, tok] += W2h[f, d]-chunks (stationary) @ silu(A^T)[f, tok]
All matmul operands are bfloat16 (fp8 DoubleRow would be ~1.44x on the PE
but every fp8-quantized operand alone costs ~2.5e-2 rel err vs the 2e-2
gate). Accumulation in fp32 PSUM; bf16 output halves the store stream.

The first supertile of slot 0 is the small remainder (quick first matmul,
short f-chains while the weight stream lands); each slot's remainder
otherwise goes last so the final drain is small. Optional PE warm-up
matmuls (MOE_WARMUP) run during the DMA lead-in to advance the HAM clock
ramp while the PE would otherwise idle.
"""

import os

import ml_dtypes
import numpy as np

B, T, D, F, E = 4, 2048, 1024, 2048, 16
N_CORES = 8
N_SLOTS = 4
P = 128
D_TILES = D // P       # 8
FH = F // 2            # 1024 intermediate per shard
FH_TILES = FH // P     # 8
W1C = 4                # w1 chunks per slot, each [P, 8, 256] (2 f-tiles)
W2C = 8                # w2 chunks per slot, each [P, 8, 128] (1 d-tile)
F_PER = FH // W1C      # 256
N_TOK = B * T          # 8192

_nc_cache = {}
last_results = None  # BassKernelResults of the most recent run (for test.py)


def _gate(x, Wg):
    """Top-2 routing. Uses the same jax ops as the reference so the discrete
    expert choice matches it bit-for-bit; falls back to float64 numpy."""
    h = np.asarray(x, dtype=np.float32).reshape(-1, D)
    try:
        import jax
        import jax.numpy as jnp

        logits = jnp.asarray(h) @ jnp.asarray(np.asarray(Wg, dtype=np.float32))
        scores, idx = jax.lax.top_k(logits, 2)
        probs = jax.nn.softmax(scores.astype(jnp.float32), axis=-1)
        return np.asarray(idx), np.asarray(probs, dtype=np.float32)
    except Exception:
        logits = h.astype(np.float64) @ np.asarray(Wg).astype(np.float64)
        idx = np.argsort(-logits, axis=1, kind="stable")[:, :2]
        s = np.take_along_axis(logits, idx, axis=1)
        s = s - s.max(axis=1, keepdims=True)
        p = np.exp(s)
        p /= p.sum(axis=1, keepdims=True)
        return idx.astype(np.int32), p.astype(np.float32)


def _plan(cap, small_first=False):
    """Supertile sizes covering cap tokens: each in [64, 512], multiple of 4,
    minimal count, exact fit (borrowing from the previous supertile when the
    remainder lands under 64). Remainder last, or first for the kernel's
    opening slot so the very first matmul chain needs minimal data."""
    cap = max(int(cap), 64)
    m = -(-cap // 4) * 4
    sizes = []
    while m > 512:
        sizes.append(512)
        m -= 512
    if m < 64 and sizes:
        sizes[-1] -= 64 - m
        m = 64
    elif m < 64:
        m = 64
    if small_first:
        return tuple([m] + sizes)
    return tuple(sizes + [m])


def _build(plans, warmup):
    """Build + compile the per-core SPMD program for per-slot supertile plans."""
    import concourse.bacc as bacc
    import concourse.mybir as mybir
    import concourse.tile as tile

    sts = [(s, S) for s in range(N_SLOTS) for S in plans[s]]
    n_st = len(sts)
    C = sum(S for _, S in sts)
    bf16 = mybir.dt.bfloat16
    f32 = mybir.dt.float32

    nc = bacc.Bacc("TRN2", target_bir_lowering=False, debug=False)
    # Tokens arrive host-packed per (supertile, dtile-pair): for supertile g
    # of size S, 4 groups of [P, 2, S]; element (p, j, s) =
    # x^T[(2q+j)*128 + p, tok_s]. Per-partition runs are 2*S*2 bytes
    # contiguous on both DRAM and SBUF sides.
    xtf = nc.dram_tensor("xtf", [D * C], bf16, kind="ExternalInput").ap()
    # weights arrive host-permuted so every chunk DMA is a contiguous copy:
    # w1[slot, chunk, p, dt, f'], w2[slot, chunk, p, ft, d']
    w1 = nc.dram_tensor("w1", [N_SLOTS, W1C, P, D_TILES, F_PER], bf16,
                        kind="ExternalInput").ap()
    w2 = nc.dram_tensor("w2", [N_SLOTS, W2C, P, FH_TILES, P], bf16,
                        kind="ExternalInput").ap()
    # bf16 output halves the store stream; host combines in fp32
    out = nc.dram_tensor("out", [D, C], bf16, kind="ExternalOutput").ap()
    fence = nc.dram_tensor("fence_scratch", [1, 1], bf16).ap()

    out_v = out.rearrange("(dt p) c -> p dt c", p=P)

    # DRAM offset of each supertile's packed token block
    xt_off = []
    o = 0
    for _, S in sts:
        xt_off.append(o)
        o += D * S

    def xt_group(g, q):
        """AP for dtile-pair group q of global supertile g: [P, 2, S_g]."""
        _, S = sts[g]
        off = xt_off[g] + q * 2 * P * S
        return xtf[off: off + 2 * P * S].rearrange(
            "(p two s) -> p two s", p=P, two=2
        )

    with tile.TileContext(nc) as tc:
        with (
            tc.tile_pool(name="wpool", bufs=1) as wpool,
            tc.tile_pool(name="xpool", bufs=3) as xpool,
            tc.tile_pool(name="apool", bufs=1) as apool,
            tc.tile_pool(name="opool", bufs=6) as opool,
            tc.tile_pool(name="ps1", bufs=3, space="PSUM") as ps1p,
            tc.tile_pool(name="ps2", bufs=5, space="PSUM") as ps2p,
        ):
            # All 48 weight tiles resident (16 MB of the 24 MB SBUF).
            w1_t = [
                [wpool.tile([P, D_TILES, F_PER], bf16, tag=f"w1_{s}_{i}", name=f"w1_{s}_{i}")
                 for i in range(W1C)]
                for s in range(N_SLOTS)
            ]
            w2_t = [
                [wpool.tile([P, FH_TILES, P], bf16, tag=f"w2_{s}_{j}", name=f"w2_{s}_{j}")
                 for j in range(W2C)]
                for s in range(N_SLOTS)
            ]

            if warmup:
                # PE warm-up during the DMA lead-in: the HAM clock ramp
                # advances with PE busy time, so dummy accumulates (into a
                # PSUM bank that rotates back before real use) while the
                # first weight chunks stream are free wall-clock.
                wu = wpool.tile([P, P], bf16, tag="wu", name="wu")
                nc.vector.memset(wu[:], 1.0)
                wps = ps1p.tile([P, P], f32, tag="ps1", name="wps")
                for i in range(warmup):
                    nc.tensor.matmul(
                        wps[:], wu[:], wu[:],
                        start=(i == 0), stop=(i == warmup - 1),
                    )

            # Slot 0's weights + first tokens stream on the sync queue in
            # exact consumption order: quarter-chunks of w1 chunk 0
            # alternate with the token dtile-pair group each piece unblocks
            # (the startup DMA path is descriptor-rate-bound, so the first
            # matmul should depend on as few KB as possible). Slots 1-3
            # stream later, each gated behind a fence read of an earlier
            # slot's silu output so the bulk weight traffic can't starve
            # the early token prefetches (emitted inside the main loop).
            S0 = sts[0][1]
            xt_first = xpool.tile([P, D_TILES, 512], bf16, tag="xt", name="xt0")
            for q in range(4):
                nc.sync.dma_start(
                    w1_t[0][0][:, 2 * q:2 * q + 2, :],
                    w1[0, 0, :, 2 * q:2 * q + 2, :],
                )
                nc.sync.dma_start(
                    xt_first[:, 2 * q:2 * q + 2, :S0], xt_group(0, q)
                )
            for i in range(1, W1C):
                nc.sync.dma_start(w1_t[0][i][:], w1[0, i])
            for j in range(W2C):
                nc.sync.dma_start(w2_t[0][j][:], w2[0, j])

            # slot s's weights are released on the sync queue once supertile
            # gate_g[s] has produced its first silu: early enough to beat
            # slot s's compute by tens of us, late enough to keep the first
            # ~25 us of DMA bandwidth for slot 0's stream + token prefetch.
            first_g = [sum(len(plans[t]) for t in range(s)) for s in range(N_SLOTS)]
            gate_g = {}
            for s in range(1, N_SLOTS):
                gg = min(first_g[s - 1] + 1, first_g[s] - 1) if s == 1 \
                    else first_g[s - 1]
                gate_g.setdefault(gg, []).append(s)

            off = 0
            xt_tiles = {0: xt_first}
            for g, (s, S) in enumerate(sts):
                xt_t = xt_tiles.pop(g)
                at = apool.tile([P, FH_TILES, 512], bf16, tag="at", name=f"at_{g}")
                for f in range(FH_TILES):
                    ps = ps1p.tile([P, 512], f32, tag="ps1", name=f"ps1_{g}_{f}")
                    for d in range(D_TILES):
                        nc.tensor.matmul(
                            ps[:, :S],
                            w1_t[s][f // 2][:, d, (f % 2) * P:(f % 2 + 1) * P],
                            xt_t[:, d, :S],
                            start=(d == 0),
                            stop=(d == D_TILES - 1),
                        )
                    nc.scalar.activation(
                        at[:, f, :S], ps[:, :S],
                        mybir.ActivationFunctionType.Silu,
                    )
                    if f == 0 and g in gate_g:
                        # Release gated slots' weight streams: a 4-byte
                        # fence read of the silu output just produced
                        # head-blocks the sync queue until compute reaches
                        # this point, keeping the early DMA bandwidth for
                        # startup + token prefetch.
                        for ss in gate_g[g]:
                            nc.sync.dma_start(fence[:], at[0:1, 0, 0:1])
                            for i in range(W1C):
                                nc.sync.dma_start(w1_t[ss][i][:], w1[ss, i])
                            for j in range(W2C):
                                nc.sync.dma_start(w2_t[ss][j][:], w2[ss, j])
                    if f == 1:
                        # Token prefetch, two supertiles deep (xpool bufs=3),
                        # on the gpsimd SWDGE queue. Triggered at f==1 so
                        # small supertiles still give the stream a wide
                        # window. The first prefetches (fresh buffers, no
                        # WAR) would otherwise stream during the startup
                        # weight crunch, so a 4-byte fence read of this
                        # supertile's f==1 silu output head-blocks the
                        # queue until mm1 of supertile 0 is well underway.
                        want = [g + 1, g + 2] if g == 0 else [g + 2]
                        want = [gg for gg in want if gg < n_st]
                        if g == 0 and want:
                            nc.gpsimd.dma_start(fence[:], at[0:1, 1, 0:1])
                        for gg in want:
                            S_next = sts[gg][1]
                            xt_n = xpool.tile([P, D_TILES, 512], bf16,
                                              tag="xt", name=f"xt_{gg}")
                            xt_tiles[gg] = xt_n
                            for q in range(4):
                                nc.gpsimd.dma_start(
                                    xt_n[:, 2 * q:2 * q + 2, :S_next],
                                    xt_group(gg, q),
                                )
                for dt in range(D_TILES):
                    ps = ps2p.tile([P, 512], f32, tag="ps2", name=f"ps2_{g}_{dt}")
                    for ft in range(FH_TILES):
                        nc.tensor.matmul(
                            ps[:, :S],
                            w2_t[s][dt][:, ft, :],
                            at[:, ft, :S],
                            start=(ft == 0),
                            stop=(ft == FH_TILES - 1),
                        )
                    ot = opool.tile([P, 512], bf16, tag="ot", name=f"ot_{g}_{dt}")
                    nc.vector.tensor_copy(ot[:, :S], ps[:, :S])
                    nc.scalar.dma_start(out_v[:, dt, off:off + S], ot[:, :S])
                off += S
    nc.compile()
    return nc


def kernel(x, Wg, W1, W2):
    global last_results
    import concourse.bass_utils as bass_utils

    x = np.asarray(x, dtype=np.float32)
    W1 = np.asarray(W1, dtype=np.float32)
    W2 = np.asarray(W2, dtype=np.float32)

    idx, probs = _gate(x, Wg)
    h = x.reshape(-1, D)

    counts = np.bincount(idx.ravel(), minlength=E)
    order = np.argsort(-counts, kind="stable")
    # 32 half-shards sorted by count; position j = slot j//8, core j%8;
    # expert order[j//2], F-half j%2. Slot capacities are then the
    # 1st/5th/9th/13th largest counts -- the optimum for any 8x4 grid.
    caps = [int(counts[order[4 * s]]) for s in range(N_SLOTS)]
    plans = tuple(
        _plan(caps[s], small_first=(s == 0)) for s in range(N_SLOTS)
    )
    slot_sum = [sum(p) for p in plans]
    slot_off = np.concatenate([[0], np.cumsum(slot_sum)])
    C = int(slot_off[-1])

    warmup = int(os.environ.get("MOE_WARMUP", "0"))
    key = (plans, warmup)
    nc = _nc_cache.get(key)
    if nc is None:
        nc = _build(plans, warmup)
        _nc_cache[key] = nc

    bf16 = ml_dtypes.bfloat16
    # token lists per expert (same order for both shard placements)
    toks = [np.nonzero((idx[:, 0] == e) | (idx[:, 1] == e))[0] for e in range(E)]

    # pos[t, j, shard] = global output column of token t's j-th expert choice
    # in shard 0/1 of that expert
    pos = np.empty((N_TOK, 2, 2), np.int64)
    for m in range(E):
        e = int(order[m])
        tok = toks[e]
        r = np.arange(len(tok))
        first = idx[tok, 0] == e
        for half, j in enumerate((2 * m, 2 * m + 1)):
            s, k = j // 8, j % 8
            gcol = k * C + slot_off[s] + r
            pos[tok[first], 0, half] = gcol[first]
            pos[tok[~first], 1, half] = gcol[~first]

    in_maps = []
    for k in range(N_CORES):
        w1c = np.empty((N_SLOTS, W1C, P, D_TILES, F_PER), np.float32)
        w2c = np.empty((N_SLOTS, W2C, P, FH_TILES, P), np.float32)
        xtf = np.zeros(D * C, np.float32)
        for s in range(N_SLOTS):
            j = 8 * s + k
            e = int(order[j // 2])
            half = j % 2
            # w1 half-shard [D, FH] -> [chunk, p, dtile, f'] contiguous chunks
            w1h = W1[e][:, half * FH:(half + 1) * FH]
            w1c[s] = w1h.reshape(D_TILES, P, W1C, F_PER).transpose(2, 1, 0, 3)
            # w2 half-shard [FH, D] -> [chunk(dtile), p, ftile, d']
            w2h = W2[e][half * FH:(half + 1) * FH, :]
            w2c[s] = w2h.reshape(FH_TILES, P, W2C, P).transpose(2, 1, 0, 3)
            # token blocks packed per (supertile, dtile-pair)
            tok = toks[e]
            o = 0
            for si, S in enumerate(plans[s]):
                g = sum(len(plans[ss]) for ss in range(s)) + si
                blk = h[tok[o:o + S]].T          # [D, n<=S]
                n = blk.shape[1]
                dst = xtf[xt_flat_off(plans, g): xt_flat_off(plans, g) + D * S]
                dst = dst.reshape(4, P, 2, S)
                dst[:, :, :, :n] = blk.reshape(4, 2, P, n).transpose(0, 2, 1, 3)
                o += S
        in_maps.append({
            "xtf": xtf.astype(bf16),
            "w1": w1c.astype(bf16),
            "w2": w2c.astype(bf16),
        })

    trace = os.environ.get("MOE_TRACE") == "1"
    kwargs = {}
    if trace:
        kwargs = {"trace": True, "trace_cores": list(range(N_CORES))}
    res = bass_utils.run_bass_kernel_spmd(
        nc, in_maps, core_ids=list(range(N_CORES)), **kwargs
    )
    last_results = res

    out_all = np.concatenate(
        [np.asarray(r["out"]).astype(np.float32) for r in res.results], axis=1
    )  # [D, 8*C]
    y = (
        (out_all[:, pos[:, 0, 0]] + out_all[:, pos[:, 0, 1]]) * probs[:, 0]
        + (out_all[:, pos[:, 1, 0]] + out_all[:, pos[:, 1, 1]]) * probs[:, 1]
    )
    return np.ascontiguousarray(y.T).reshape(B, T, D).astype(np.float32)


def xt_flat_off(plans, g):
    """DRAM float offset of global supertile g's packed token block."""
    sizes = [S for p in plans for S in p]
    return D * sum(sizes[:g])


# revision 11
# speedup vs baseline: 1.0364x; 1.0364x over previous
"""MoE layer (top-2 routing, 16 experts) on 8 Trainium2 NeuronCores.

Strategy: tensor-parallel expert sharding (TP=2). Each expert's FFN is split
along the intermediate dimension F into two half-shards [D, F/2] / [F/2, D];
the 32 shards are laid out on an 8-core x 4-slot grid (slot capacities fixed
across cores for SPMD), sorted so slot s's capacity is the s-th order
statistic of the duplicated count list: C = (c1 + c5 + c9 + c13)/2
column-equivalents vs c1 + c9 for whole-expert placement -- less padding
waste from expert load imbalance. Every token is shipped to both shard cores
of each of its two experts; the two half-FFN partial outputs (silu applies
per-F-element, so each shard's silu(x @ W1h) @ W2h is an exact partial term)
are summed on the host during the combine.

All 4 slots' weights (16 MB bf16) stay resident in SBUF, so there is no
weight-slot recycling WAR chain: every weight DMA issues up front in
consumption order. Slot 0's W1 + first tokens stream on the sync HWDGE
queue, slot 0's W2 on the scalar HWDGE queue in parallel (startup is
DMA-descriptor-rate-bound, so two queues nearly double the early stream
rate); later slots trail on sync. Tokens prefetch one supertile ahead on the
gpsimd SWDGE queue, first prefetch fence-gated behind the first supertile's
last silu so it can't compete with the startup weight crunch.

Device layout keeps tokens on the matmul free dimension throughout (x is
shipped transposed, [D, tokens]) so no on-chip transposes are needed:
  mm1: A^T[f, tok] += W1h[d, f]^T-chunks (stationary) @ x^T[d, tok]
  silu on ScalarE, PSUM -> SBUF (bf16)
  mm2: y^T[d, tok] += W2h[f, d]-chunks (stationary) @ silu(A^T)[f, tok]
All matmul operands are bfloat16 (fp8 DoubleRow would be ~1.44x on the PE
but every fp8-quantized operand alone costs ~2.5e-2 rel err vs the 2e-2
gate). Accumulation in fp32 PSUM; bf16 output halves the store stream.

The first supertile of slot 0 is the small remainder (quick first matmul,
short f-chains while the weight stream lands); each slot's remainder
otherwise goes last so the final drain is small. Optional PE warm-up
matmuls (MOE_WARMUP) run during the DMA lead-in to advance the HAM clock
ramp while the PE would otherwise idle.
"""

import os

import ml_dtypes
import numpy as np

B, T, D, F, E = 4, 2048, 1024, 2048, 16
N_CORES = 8
N_SLOTS = 4
P = 128
D_TILES = D // P       # 8
FH = F // 2            # 1024 intermediate per shard
FH_TILES = FH // P     # 8
W1C = 4                # w1 chunks per slot, each [P, 8, 256] (2 f-tiles)
W2C = 8                # w2 chunks per slot, each [P, 8, 128] (1 d-tile)
F_PER = FH // W1C      # 256
N_TOK = B * T          # 8192

_nc_cache = {}
last_results = None  # BassKernelResults of the most recent run (for test.py)


def _gate(x, Wg):
    """Top-2 routing. Uses the same jax ops as the reference so the discrete
    expert choice matches it bit-for-bit; falls back to float64 numpy."""
    h = np.asarray(x, dtype=np.float32).reshape(-1, D)
    try:
        import jax
        import jax.numpy as jnp

        logits = jnp.asarray(h) @ jnp.asarray(np.asarray(Wg, dtype=np.float32))
        scores, idx = jax.lax.top_k(logits, 2)
        probs = jax.nn.softmax(scores.astype(jnp.float32), axis=-1)
        return np.asarray(idx), np.asarray(probs, dtype=np.float32)
    except Exception:
        logits = h.astype(np.float64) @ np.asarray(Wg).astype(np.float64)
        idx = np.argsort(-logits, axis=1, kind="stable")[:, :2]
        s = np.take_along_axis(logits, idx, axis=1)
        s = s - s.max(axis=1, keepdims=True)
        p = np.exp(s)
        p /= p.sum(axis=1, keepdims=True)
        return idx.astype(np.int32), p.astype(np.float32)


def _plan(cap, small_first=False):
    """Supertile sizes covering cap tokens: each in [64, 512], multiple of 4,
    minimal count, exact fit (borrowing from the previous supertile when the
    remainder lands under 64). Remainder last, or first for the kernel's
    opening slot so the very first matmul chain needs minimal data."""
    cap = max(int(cap), 64)
    m = -(-cap // 4) * 4
    sizes = []
    while m > 512:
        sizes.append(512)
        m -= 512
    if m < 64 and sizes:
        sizes[-1] -= 64 - m
        m = 64
    elif m < 64:
        m = 64
    if small_first:
        return tuple([m] + sizes)
    return tuple(sizes + [m])


def _build(plans, warmup):
    """Build + compile the per-core SPMD program for per-slot supertile plans."""
    import concourse.bacc as bacc
    import concourse.mybir as mybir
    import concourse.tile as tile

    sts = [(s, S) for s in range(N_SLOTS) for S in plans[s]]
    n_st = len(sts)
    C = sum(S for _, S in sts)
    bf16 = mybir.dt.bfloat16
    f32 = mybir.dt.float32

    nc = bacc.Bacc("TRN2", target_bir_lowering=False, debug=False)
    # Tokens arrive host-packed per (supertile, dtile-pair): for supertile g
    # of size S, 4 groups of [P, 2, S]; element (p, j, s) =
    # x^T[(2q+j)*128 + p, tok_s]. Per-partition runs are 2*S*2 bytes
    # contiguous on both DRAM and SBUF sides.
    xtf = nc.dram_tensor("xtf", [D * C], bf16, kind="ExternalInput").ap()
    # weights arrive host-permuted so every chunk DMA is a contiguous copy:
    # w1[slot, chunk, p, dt, f'], w2[slot, chunk, p, ft, d']
    w1 = nc.dram_tensor("w1", [N_SLOTS, W1C, P, D_TILES, F_PER], bf16,
                        kind="ExternalInput").ap()
    w2 = nc.dram_tensor("w2", [N_SLOTS, W2C, P, FH_TILES, P], bf16,
                        kind="ExternalInput").ap()
    # bf16 output halves the store stream; host combines in fp32
    out = nc.dram_tensor("out", [D, C], bf16, kind="ExternalOutput").ap()
    fence = nc.dram_tensor("fence_scratch", [1, 1], bf16).ap()

    out_v = out.rearrange("(dt p) c -> p dt c", p=P)

    # DRAM offset of each supertile's packed token block
    xt_off = []
    o = 0
    for _, S in sts:
        xt_off.append(o)
        o += D * S

    def xt_group(g, q):
        """AP for dtile-pair group q of global supertile g: [P, 2, S_g]."""
        _, S = sts[g]
        off = xt_off[g] + q * 2 * P * S
        return xtf[off: off + 2 * P * S].rearrange(
            "(p two s) -> p two s", p=P, two=2
        )

    with tile.TileContext(nc) as tc:
        with (
            tc.tile_pool(name="wpool", bufs=1) as wpool,
            tc.tile_pool(name="xpool", bufs=3) as xpool,
            tc.tile_pool(name="apool", bufs=1) as apool,
            tc.tile_pool(name="opool", bufs=6) as opool,
            tc.tile_pool(name="ps1", bufs=4, space="PSUM") as ps1p,
            tc.tile_pool(name="ps2", bufs=4, space="PSUM") as ps2p,
        ):
            # All 48 weight tiles resident (16 MB of the 24 MB SBUF).
            w1_t = [
                [wpool.tile([P, D_TILES, F_PER], bf16, tag=f"w1_{s}_{i}", name=f"w1_{s}_{i}")
                 for i in range(W1C)]
                for s in range(N_SLOTS)
            ]
            w2_t = [
                [wpool.tile([P, FH_TILES, P], bf16, tag=f"w2_{s}_{j}", name=f"w2_{s}_{j}")
                 for j in range(W2C)]
                for s in range(N_SLOTS)
            ]

            if warmup:
                # PE warm-up during the DMA lead-in: the HAM clock ramp
                # advances with PE busy time, so dummy accumulates (into a
                # PSUM bank that rotates back before real use) while the
                # first weight chunks stream are free wall-clock.
                wu = wpool.tile([P, P], bf16, tag="wu", name="wu")
                nc.vector.memset(wu[:], 1.0)
                wps = ps1p.tile([P, P], f32, tag="ps1", name="wps")
                for i in range(warmup):
                    nc.tensor.matmul(
                        wps[:], wu[:], wu[:],
                        start=(i == 0), stop=(i == warmup - 1),
                    )

            # Slot 0's weights + first tokens stream on the sync queue in
            # exact consumption order: quarter-chunks of w1 chunk 0
            # alternate with the token dtile-pair group each piece unblocks
            # (the startup DMA path is descriptor-rate-bound, so the first
            # matmul should depend on as few KB as possible). Slots 1-3
            # stream later, each gated behind a fence read of an earlier
            # slot's silu output so the bulk weight traffic can't starve
            # the early token prefetches (emitted inside the main loop).
            S0 = sts[0][1]
            xt_first = xpool.tile([P, D_TILES, 512], bf16, tag="xt", name="xt0")
            for q in range(4):
                nc.sync.dma_start(
                    w1_t[0][0][:, 2 * q:2 * q + 2, :],
                    w1[0, 0, :, 2 * q:2 * q + 2, :],
                )
                nc.scalar.dma_start(
                    xt_first[:, 2 * q:2 * q + 2, :S0], xt_group(0, q)
                )
            for i in range(1, W1C):
                nc.sync.dma_start(w1_t[0][i][:], w1[0, i])
            for j in range(W2C):
                nc.sync.dma_start(w2_t[0][j][:], w2[0, j])

            # slot s's weights are released on the sync queue once supertile
            # gate_g[s] has produced its first silu: early enough to beat
            # slot s's compute by tens of us, late enough to keep the first
            # ~25 us of DMA bandwidth for slot 0's stream + token prefetch.
            first_g = [sum(len(plans[t]) for t in range(s)) for s in range(N_SLOTS)]
            gate_g = {}
            for s in range(1, N_SLOTS):
                gg = min(first_g[s - 1] + 1, first_g[s] - 1) if s == 1 \
                    else first_g[s - 1]
                gate_g.setdefault(gg, []).append(s)

            off = 0
            xt_tiles = {0: xt_first}
            for g, (s, S) in enumerate(sts):
                xt_t = xt_tiles.pop(g)
                at = apool.tile([P, FH_TILES, 512], bf16, tag="at", name=f"at_{g}")
                for f in range(FH_TILES):
                    ps = ps1p.tile([P, 512], f32, tag="ps1", name=f"ps1_{g}_{f}")
                    for d in range(D_TILES):
                        nc.tensor.matmul(
                            ps[:, :S],
                            w1_t[s][f // 2][:, d, (f % 2) * P:(f % 2 + 1) * P],
                            xt_t[:, d, :S],
                            start=(d == 0),
                            stop=(d == D_TILES - 1),
                        )
                    nc.scalar.activation(
                        at[:, f, :S], ps[:, :S],
                        mybir.ActivationFunctionType.Silu,
                    )
                    if f == 0 and g in gate_g:
                        # Release gated slots' weight streams: a 4-byte
                        # fence read of the silu output just produced
                        # head-blocks the sync queue until compute reaches
                        # this point, keeping the early DMA bandwidth for
                        # startup + token prefetch.
                        for ss in gate_g[g]:
                            nc.sync.dma_start(fence[:], at[0:1, 0, 0:1])
                            for i in range(W1C):
                                nc.sync.dma_start(w1_t[ss][i][:], w1[ss, i])
                            for j in range(W2C):
                                nc.sync.dma_start(w2_t[ss][j][:], w2[ss, j])
                    trigger = (FH_TILES - 1) if g == 0 else 1
                    if f == trigger:
                        # Token prefetch, two supertiles deep (xpool bufs=3),
                        # on the gpsimd SWDGE queue. Triggered at f==1 so
                        # small supertiles still give the stream a wide
                        # window. The first prefetches (fresh buffers, no
                        # WAR) would otherwise stream during the startup
                        # weight crunch, so a 4-byte fence read of supertile
                        # 0's last silu output head-blocks the queue until
                        # mm1 of supertile 0 completes.
                        want = [g + 1, g + 2] if g == 0 else [g + 2]
                        want = [gg for gg in want if gg < n_st]
                        if g == 0 and want:
                            nc.gpsimd.dma_start(fence[:], at[0:1, trigger, 0:1])
                        for gg in want:
                            S_next = sts[gg][1]
                            xt_n = xpool.tile([P, D_TILES, 512], bf16,
                                              tag="xt", name=f"xt_{gg}")
                            xt_tiles[gg] = xt_n
                            for q in range(4):
                                nc.gpsimd.dma_start(
                                    xt_n[:, 2 * q:2 * q + 2, :S_next],
                                    xt_group(gg, q),
                                )
                for dt in range(D_TILES):
                    ps = ps2p.tile([P, 512], f32, tag="ps2", name=f"ps2_{g}_{dt}")
                    for ft in range(FH_TILES):
                        nc.tensor.matmul(
                            ps[:, :S],
                            w2_t[s][dt][:, ft, :],
                            at[:, ft, :S],
                            start=(ft == 0),
                            stop=(ft == FH_TILES - 1),
                        )
                    ot = opool.tile([P, 512], bf16, tag="ot", name=f"ot_{g}_{dt}")
                    nc.vector.tensor_copy(ot[:, :S], ps[:, :S])
                    nc.sync.dma_start(out_v[:, dt, off:off + S], ot[:, :S])
                off += S
    nc.compile()
    return nc


def kernel(x, Wg, W1, W2):
    global last_results
    import concourse.bass_utils as bass_utils

    x = np.asarray(x, dtype=np.float32)
    W1 = np.asarray(W1, dtype=np.float32)
    W2 = np.asarray(W2, dtype=np.float32)

    idx, probs = _gate(x, Wg)
    h = x.reshape(-1, D)

    counts = np.bincount(idx.ravel(), minlength=E)
    order = np.argsort(-counts, kind="stable")
    # 32 half-shards sorted by count; position j = slot j//8, core j%8;
    # expert order[j//2], F-half j%2. Slot capacities are then the
    # 1st/5th/9th/13th largest counts -- the optimum for any 8x4 grid.
    caps = [int(counts[order[4 * s]]) for s in range(N_SLOTS)]
    plans = tuple(
        _plan(caps[s], small_first=(s == 0)) for s in range(N_SLOTS)
    )
    slot_sum = [sum(p) for p in plans]
    slot_off = np.concatenate([[0], np.cumsum(slot_sum)])
    C = int(slot_off[-1])

    warmup = int(os.environ.get("MOE_WARMUP", "0"))
    key = (plans, warmup)
    nc = _nc_cache.get(key)
    if nc is None:
        nc = _build(plans, warmup)
        _nc_cache[key] = nc

    bf16 = ml_dtypes.bfloat16
    # token lists per expert (same order for both shard placements)
    toks = [np.nonzero((idx[:, 0] == e) | (idx[:, 1] == e))[0] for e in range(E)]

    # pos[t, j, shard] = global output column of token t's j-th expert choice
    # in shard 0/1 of that expert
    pos = np.empty((N_TOK, 2, 2), np.int64)
    for m in range(E):
        e = int(order[m])
        tok = toks[e]
        r = np.arange(len(tok))
        first = idx[tok, 0] == e
        for half, j in enumerate((2 * m, 2 * m + 1)):
            s, k = j // 8, j % 8
            gcol = k * C + slot_off[s] + r
            pos[tok[first], 0, half] = gcol[first]
            pos[tok[~first], 1, half] = gcol[~first]

    in_maps = []
    for k in range(N_CORES):
        w1c = np.empty((N_SLOTS, W1C, P, D_TILES, F_PER), np.float32)
        w2c = np.empty((N_SLOTS, W2C, P, FH_TILES, P), np.float32)
        xtf = np.zeros(D * C, np.float32)
        for s in range(N_SLOTS):
            j = 8 * s + k
            e = int(order[j // 2])
            half = j % 2
            # w1 half-shard [D, FH] -> [chunk, p, dtile, f'] contiguous chunks
            w1h = W1[e][:, half * FH:(half + 1) * FH]
            w1c[s] = w1h.reshape(D_TILES, P, W1C, F_PER).transpose(2, 1, 0, 3)
            # w2 half-shard [FH, D] -> [chunk(dtile), p, ftile, d']
            w2h = W2[e][half * FH:(half + 1) * FH, :]
            w2c[s] = w2h.reshape(FH_TILES, P, W2C, P).transpose(2, 1, 0, 3)
            # token blocks packed per (supertile, dtile-pair)
            tok = toks[e]
            o = 0
            for si, S in enumerate(plans[s]):
                g = sum(len(plans[ss]) for ss in range(s)) + si
                blk = h[tok[o:o + S]].T          # [D, n<=S]
                n = blk.shape[1]
                dst = xtf[xt_flat_off(plans, g): xt_flat_off(plans, g) + D * S]
                dst = dst.reshape(4, P, 2, S)
                dst[:, :, :, :n] = blk.reshape(4, 2, P, n).transpose(0, 2, 1, 3)
                o += S
        in_maps.append({
            "xtf": xtf.astype(bf16),
            "w1": w1c.astype(bf16),
            "w2": w2c.astype(bf16),
        })

    trace = os.environ.get("MOE_TRACE") == "1"
    kwargs = {}
    if trace:
        kwargs = {"trace": True, "trace_cores": list(range(N_CORES))}
    res = bass_utils.run_bass_kernel_spmd(
        nc, in_maps, core_ids=list(range(N_CORES)), **kwargs
    )
    last_results = res

    out_all = np.concatenate(
        [np.asarray(r["out"]).astype(np.float32) for r in res.results], axis=1
    )  # [D, 8*C]
    y = (
        (out_all[:, pos[:, 0, 0]] + out_all[:, pos[:, 0, 1]]) * probs[:, 0]
        + (out_all[:, pos[:, 1, 0]] + out_all[:, pos[:, 1, 1]]) * probs[:, 1]
    )
    return np.ascontiguousarray(y.T).reshape(B, T, D).astype(np.float32)


def xt_flat_off(plans, g):
    """DRAM float offset of global supertile g's packed token block."""
    sizes = [S for p in plans for S in p]
    return D * sum(sizes[:g])


# revision 12
# speedup vs baseline: 1.0456x; 1.0089x over previous
"""MoE layer (top-2 routing, 16 experts) on 8 Trainium2 NeuronCores.

Strategy: tensor-parallel expert sharding (TP=2). Each expert's FFN is split
along the intermediate dimension F into two half-shards [D, F/2] / [F/2, D];
the 32 shards are laid out on an 8-core x 4-slot grid (slot capacities fixed
across cores for SPMD), sorted so slot s's capacity is the s-th order
statistic of the duplicated count list: C = (c1 + c5 + c9 + c13)/2
column-equivalents vs c1 + c9 for whole-expert placement -- less padding
waste from expert load imbalance. Every token is shipped to both shard cores
of each of its two experts; the two half-FFN partial outputs (silu applies
per-F-element, so each shard's silu(x @ W1h) @ W2h is an exact partial term)
are summed on the host during the combine.

All 4 slots' weights (16 MB bf16) stay resident in SBUF, so there is no
weight-slot recycling WAR chain: every weight DMA issues up front in
consumption order. Slot 0's W1 + first tokens stream on the sync HWDGE
queue, slot 0's W2 on the scalar HWDGE queue in parallel (startup is
DMA-descriptor-rate-bound, so two queues nearly double the early stream
rate); later slots trail on sync. Tokens prefetch one supertile ahead on the
gpsimd SWDGE queue, first prefetch fence-gated behind the first supertile's
last silu so it can't compete with the startup weight crunch.

Device layout keeps tokens on the matmul free dimension throughout (x is
shipped transposed, [D, tokens]) so no on-chip transposes are needed:
  mm1: A^T[f, tok] += W1h[d, f]^T-chunks (stationary) @ x^T[d, tok]
  silu on ScalarE, PSUM -> SBUF (bf16)
  mm2: y^T[d, tok] += W2h[f, d]-chunks (stationary) @ silu(A^T)[f, tok]
All matmul operands are bfloat16 (fp8 DoubleRow would be ~1.44x on the PE
but every fp8-quantized operand alone costs ~2.5e-2 rel err vs the 2e-2
gate). Accumulation in fp32 PSUM; bf16 output halves the store stream.

The first supertile of slot 0 is the small remainder (quick first matmul,
short f-chains while the weight stream lands); each slot's remainder
otherwise goes last so the final drain is small. Optional PE warm-up
matmuls (MOE_WARMUP) run during the DMA lead-in to advance the HAM clock
ramp while the PE would otherwise idle.
"""

import os

import ml_dtypes
import numpy as np

B, T, D, F, E = 4, 2048, 1024, 2048, 16
N_CORES = 8
N_SLOTS = 4
P = 128
D_TILES = D // P       # 8
FH = F // 2            # 1024 intermediate per shard
FH_TILES = FH // P     # 8
W1C = 4                # w1 chunks per slot, each [P, 8, 256] (2 f-tiles)
W2C = 8                # w2 chunks per slot, each [P, 8, 128] (1 d-tile)
F_PER = FH // W1C      # 256
N_TOK = B * T          # 8192

_nc_cache = {}
last_results = None  # BassKernelResults of the most recent run (for test.py)


def _gate(x, Wg):
    """Top-2 routing. Uses the same jax ops as the reference so the discrete
    expert choice matches it bit-for-bit; falls back to float64 numpy."""
    h = np.asarray(x, dtype=np.float32).reshape(-1, D)
    try:
        import jax
        import jax.numpy as jnp

        logits = jnp.asarray(h) @ jnp.asarray(np.asarray(Wg, dtype=np.float32))
        scores, idx = jax.lax.top_k(logits, 2)
        probs = jax.nn.softmax(scores.astype(jnp.float32), axis=-1)
        return np.asarray(idx), np.asarray(probs, dtype=np.float32)
    except Exception:
        logits = h.astype(np.float64) @ np.asarray(Wg).astype(np.float64)
        idx = np.argsort(-logits, axis=1, kind="stable")[:, :2]
        s = np.take_along_axis(logits, idx, axis=1)
        s = s - s.max(axis=1, keepdims=True)
        p = np.exp(s)
        p /= p.sum(axis=1, keepdims=True)
        return idx.astype(np.int32), p.astype(np.float32)


def _plan(cap, small_first=False):
    """Supertile sizes covering cap tokens: each in [64, 512], multiple of 4,
    minimal count, exact fit (borrowing from the previous supertile when the
    remainder lands under 64). Remainder last, or first for the kernel's
    opening slot so the very first matmul chain needs minimal data."""
    cap = max(int(cap), 64)
    m = -(-cap // 4) * 4
    sizes = []
    while m > 512:
        sizes.append(512)
        m -= 512
    if m < 64 and sizes:
        sizes[-1] -= 64 - m
        m = 64
    elif m < 64:
        m = 64
    if small_first:
        return tuple([m] + sizes)
    return tuple(sizes + [m])


def _build(plans, warmup):
    """Build + compile the per-core SPMD program for per-slot supertile plans."""
    import concourse.bacc as bacc
    import concourse.mybir as mybir
    import concourse.tile as tile

    sts = [(s, S) for s in range(N_SLOTS) for S in plans[s]]
    n_st = len(sts)
    C = sum(S for _, S in sts)
    bf16 = mybir.dt.bfloat16
    f32 = mybir.dt.float32

    nc = bacc.Bacc("TRN2", target_bir_lowering=False, debug=False)
    # Tokens arrive host-packed per (supertile, dtile-pair): for supertile g
    # of size S, 4 groups of [P, 2, S]; element (p, j, s) =
    # x^T[(2q+j)*128 + p, tok_s]. Per-partition runs are 2*S*2 bytes
    # contiguous on both DRAM and SBUF sides.
    xtf = nc.dram_tensor("xtf", [D * C], bf16, kind="ExternalInput").ap()
    # weights arrive host-permuted so every chunk DMA is a contiguous copy:
    # w1[slot, chunk, p, dt, f'], w2[slot, chunk, p, ft, d']
    w1 = nc.dram_tensor("w1", [N_SLOTS, W1C, P, D_TILES, F_PER], bf16,
                        kind="ExternalInput").ap()
    w2 = nc.dram_tensor("w2", [N_SLOTS, W2C, P, FH_TILES, P], bf16,
                        kind="ExternalInput").ap()
    # bf16 output halves the store stream; host combines in fp32
    out = nc.dram_tensor("out", [D, C], bf16, kind="ExternalOutput").ap()
    fence = nc.dram_tensor("fence_scratch", [1, 1], bf16).ap()

    out_v = out.rearrange("(dt p) c -> p dt c", p=P)

    # DRAM offset of each supertile's packed token block
    xt_off = []
    o = 0
    for _, S in sts:
        xt_off.append(o)
        o += D * S

    def xt_group(g, q):
        """AP for dtile-pair group q of global supertile g: [P, 2, S_g]."""
        _, S = sts[g]
        off = xt_off[g] + q * 2 * P * S
        return xtf[off: off + 2 * P * S].rearrange(
            "(p two s) -> p two s", p=P, two=2
        )

    with tile.TileContext(nc) as tc:
        with (
            tc.tile_pool(name="wpool", bufs=1) as wpool,
            tc.tile_pool(name="xpool", bufs=3) as xpool,
            tc.tile_pool(name="apool", bufs=1) as apool,
            tc.tile_pool(name="opool", bufs=6) as opool,
            tc.tile_pool(name="ps1", bufs=4, space="PSUM") as ps1p,
            tc.tile_pool(name="ps2", bufs=4, space="PSUM") as ps2p,
        ):
            # All 48 weight tiles resident (16 MB of the 24 MB SBUF).
            w1_t = [
                [wpool.tile([P, D_TILES, F_PER], bf16, tag=f"w1_{s}_{i}", name=f"w1_{s}_{i}")
                 for i in range(W1C)]
                for s in range(N_SLOTS)
            ]
            w2_t = [
                [wpool.tile([P, FH_TILES, P], bf16, tag=f"w2_{s}_{j}", name=f"w2_{s}_{j}")
                 for j in range(W2C)]
                for s in range(N_SLOTS)
            ]

            if warmup:
                # PE warm-up during the DMA lead-in: the HAM clock ramp
                # advances with PE busy time, so dummy accumulates (into a
                # PSUM bank that rotates back before real use) while the
                # first weight chunks stream are free wall-clock.
                wu = wpool.tile([P, P], bf16, tag="wu", name="wu")
                nc.vector.memset(wu[:], 1.0)
                wps = ps1p.tile([P, P], f32, tag="ps1", name="wps")
                for i in range(warmup):
                    nc.tensor.matmul(
                        wps[:], wu[:], wu[:],
                        start=(i == 0), stop=(i == warmup - 1),
                    )

            # Slot 0's weights + first tokens stream on the sync queue in
            # exact consumption order: quarter-chunks of w1 chunk 0
            # alternate with the token dtile-pair group each piece unblocks
            # (the startup DMA path is descriptor-rate-bound, so the first
            # matmul should depend on as few KB as possible). Slots 1-3
            # stream later, each gated behind a fence read of an earlier
            # slot's silu output so the bulk weight traffic can't starve
            # the early token prefetches (emitted inside the main loop).
            S0 = sts[0][1]
            xt_first = xpool.tile([P, D_TILES, 512], bf16, tag="xt", name="xt0")
            for q in range(4):
                nc.sync.dma_start(
                    w1_t[0][0][:, 2 * q:2 * q + 2, :],
                    w1[0, 0, :, 2 * q:2 * q + 2, :],
                )
                nc.scalar.dma_start(
                    xt_first[:, 2 * q:2 * q + 2, :S0], xt_group(0, q)
                )
            for i in range(1, W1C):
                nc.sync.dma_start(w1_t[0][i][:], w1[0, i])
            for j in range(W2C):
                nc.sync.dma_start(w2_t[0][j][:], w2[0, j])

            # slot s's weights are released on the sync queue once supertile
            # gate_g[s] has produced its first silu: early enough to beat
            # slot s's compute by tens of us, late enough to keep the first
            # ~25 us of DMA bandwidth for slot 0's stream + token prefetch.
            first_g = [sum(len(plans[t]) for t in range(s)) for s in range(N_SLOTS)]
            gate_g = {}
            for s in range(1, N_SLOTS):
                gg = min(first_g[s - 1] + 1, first_g[s] - 1) if s == 1 \
                    else first_g[s - 1]
                gate_g.setdefault(gg, []).append(s)

            off = 0
            xt_tiles = {0: xt_first}
            for g, (s, S) in enumerate(sts):
                xt_t = xt_tiles.pop(g)
                at = apool.tile([P, FH_TILES, 512], bf16, tag="at", name=f"at_{g}")
                for f in range(FH_TILES):
                    ps = ps1p.tile([P, 512], f32, tag="ps1", name=f"ps1_{g}_{f}")
                    for d in range(D_TILES):
                        nc.tensor.matmul(
                            ps[:, :S],
                            w1_t[s][f // 2][:, d, (f % 2) * P:(f % 2 + 1) * P],
                            xt_t[:, d, :S],
                            start=(d == 0),
                            stop=(d == D_TILES - 1),
                        )
                    nc.scalar.activation(
                        at[:, f, :S], ps[:, :S],
                        mybir.ActivationFunctionType.Silu,
                    )
                    if f == 0 and g in gate_g:
                        # Release gated slots' weight streams: a 4-byte
                        # fence read of the silu output just produced
                        # head-blocks the sync queue until compute reaches
                        # this point, keeping the early DMA bandwidth for
                        # startup + token prefetch.
                        for ss in gate_g[g]:
                            nc.sync.dma_start(fence[:], at[0:1, 0, 0:1])
                            for i in range(W1C):
                                nc.sync.dma_start(w1_t[ss][i][:], w1[ss, i])
                            for j in range(W2C):
                                nc.sync.dma_start(w2_t[ss][j][:], w2[ss, j])
                    if g == 0:
                        # Supertile 0 seeds the 2-deep prefetch pipeline:
                        # g1's tokens release at f==1 (fence-gated so they
                        # trail slot 0's weight stream), g2's at the last
                        # f-tile.
                        want = [g + 1] if f == 1 else (
                            [g + 2] if f == FH_TILES - 1 else [])
                    else:
                        want = [g + 2] if f == 1 else []
                    if want:
                        # Token prefetch, two supertiles deep (xpool bufs=3),
                        # on the gpsimd SWDGE queue. Triggered at f==1 so
                        # small supertiles still give the stream a wide
                        # window before the next mm1 needs the data.
                        want = [gg for gg in want if gg < n_st]
                        if g == 0 and f == 1 and want:
                            nc.gpsimd.dma_start(fence[:], at[0:1, 1, 0:1])
                        for gg in want:
                            S_next = sts[gg][1]
                            xt_n = xpool.tile([P, D_TILES, 512], bf16,
                                              tag="xt", name=f"xt_{gg}")
                            xt_tiles[gg] = xt_n
                            for q in range(4):
                                nc.gpsimd.dma_start(
                                    xt_n[:, 2 * q:2 * q + 2, :S_next],
                                    xt_group(gg, q),
                                )
                for dt in range(D_TILES):
                    ps = ps2p.tile([P, 512], f32, tag="ps2", name=f"ps2_{g}_{dt}")
                    for ft in range(FH_TILES):
                        nc.tensor.matmul(
                            ps[:, :S],
                            w2_t[s][dt][:, ft, :],
                            at[:, ft, :S],
                            start=(ft == 0),
                            stop=(ft == FH_TILES - 1),
                        )
                    ot = opool.tile([P, 512], bf16, tag="ot", name=f"ot_{g}_{dt}")
                    nc.vector.tensor_copy(ot[:, :S], ps[:, :S])
                    nc.sync.dma_start(out_v[:, dt, off:off + S], ot[:, :S])
                off += S
    nc.compile()
    return nc


def kernel(x, Wg, W1, W2):
    global last_results
    import concourse.bass_utils as bass_utils

    x = np.asarray(x, dtype=np.float32)
    W1 = np.asarray(W1, dtype=np.float32)
    W2 = np.asarray(W2, dtype=np.float32)

    idx, probs = _gate(x, Wg)
    h = x.reshape(-1, D)

    counts = np.bincount(idx.ravel(), minlength=E)
    order = np.argsort(-counts, kind="stable")
    # 32 half-shards sorted by count; position j = slot j//8, core j%8;
    # expert order[j//2], F-half j%2. Slot capacities are then the
    # 1st/5th/9th/13th largest counts -- the optimum for any 8x4 grid.
    caps = [int(counts[order[4 * s]]) for s in range(N_SLOTS)]
    plans = tuple(_plan(caps[s]) for s in range(N_SLOTS))
    slot_sum = [sum(p) for p in plans]
    slot_off = np.concatenate([[0], np.cumsum(slot_sum)])
    C = int(slot_off[-1])

    warmup = int(os.environ.get("MOE_WARMUP", "0"))
    key = (plans, warmup)
    nc = _nc_cache.get(key)
    if nc is None:
        nc = _build(plans, warmup)
        _nc_cache[key] = nc

    bf16 = ml_dtypes.bfloat16
    # token lists per expert (same order for both shard placements)
    toks = [np.nonzero((idx[:, 0] == e) | (idx[:, 1] == e))[0] for e in range(E)]

    # pos[t, j, shard] = global output column of token t's j-th expert choice
    # in shard 0/1 of that expert
    pos = np.empty((N_TOK, 2, 2), np.int64)
    for m in range(E):
        e = int(order[m])
        tok = toks[e]
        r = np.arange(len(tok))
        first = idx[tok, 0] == e
        for half, j in enumerate((2 * m, 2 * m + 1)):
            s, k = j // 8, j % 8
            gcol = k * C + slot_off[s] + r
            pos[tok[first], 0, half] = gcol[first]
            pos[tok[~first], 1, half] = gcol[~first]

    in_maps = []
    for k in range(N_CORES):
        w1c = np.empty((N_SLOTS, W1C, P, D_TILES, F_PER), np.float32)
        w2c = np.empty((N_SLOTS, W2C, P, FH_TILES, P), np.float32)
        xtf = np.zeros(D * C, np.float32)
        for s in range(N_SLOTS):
            j = 8 * s + k
            e = int(order[j // 2])
            half = j % 2
            # w1 half-shard [D, FH] -> [chunk, p, dtile, f'] contiguous chunks
            w1h = W1[e][:, half * FH:(half + 1) * FH]
            w1c[s] = w1h.reshape(D_TILES, P, W1C, F_PER).transpose(2, 1, 0, 3)
            # w2 half-shard [FH, D] -> [chunk(dtile), p, ftile, d']
            w2h = W2[e][half * FH:(half + 1) * FH, :]
            w2c[s] = w2h.reshape(FH_TILES, P, W2C, P).transpose(2, 1, 0, 3)
            # token blocks packed per (supertile, dtile-pair)
            tok = toks[e]
            o = 0
            for si, S in enumerate(plans[s]):
                g = sum(len(plans[ss]) for ss in range(s)) + si
                blk = h[tok[o:o + S]].T          # [D, n<=S]
                n = blk.shape[1]
                dst = xtf[xt_flat_off(plans, g): xt_flat_off(plans, g) + D * S]
                dst = dst.reshape(4, P, 2, S)
                dst[:, :, :, :n] = blk.reshape(4, 2, P, n).transpose(0, 2, 1, 3)
                o += S
        in_maps.append({
            "xtf": xtf.astype(bf16),
            "w1": w1c.astype(bf16),
            "w2": w2c.astype(bf16),
        })

    trace = os.environ.get("MOE_TRACE") == "1"
    kwargs = {}
    if trace:
        kwargs = {"trace": True, "trace_cores": list(range(N_CORES))}
    res = bass_utils.run_bass_kernel_spmd(
        nc, in_maps, core_ids=list(range(N_CORES)), **kwargs
    )
    last_results = res

    out_all = np.concatenate(
        [np.asarray(r["out"]).astype(np.float32) for r in res.results], axis=1
    )  # [D, 8*C]
    y = (
        (out_all[:, pos[:, 0, 0]] + out_all[:, pos[:, 0, 1]]) * probs[:, 0]
        + (out_all[:, pos[:, 1, 0]] + out_all[:, pos[:, 1, 1]]) * probs[:, 1]
    )
    return np.ascontiguousarray(y.T).reshape(B, T, D).astype(np.float32)


def xt_flat_off(plans, g):
    """DRAM float offset of global supertile g's packed token block."""
    sizes = [S for p in plans for S in p]
    return D * sum(sizes[:g])
